# revision 11
# baseline (speedup 1.0000x reference)
"""Head-parallel GQA attention kernel for 8 TRN2 NeuronCores.

Sharding: core i owns KV head i and Q heads (2i, 2i+1), plus the matching
256-column slice of wo's input dim. Each core computes a partial output
(its heads' contribution through wo); the host sums the 8 partials.

All device compute is bf16 (PSUM accumulation in f32). The host pre-bakes
layouts so the device never rearranges inputs:
  - All DRAM inputs are partition-major [128, ...] with contiguous
    per-partition lines, so every load is ~128 fat descriptors.
  - wq/wk rows are permuted per-head (evens then odds) so interleaved-pair
    RoPE becomes rotate-half form: pairs live in partition halves [0:64]
    and [64:128] of the projected Q^T/K^T tiles. RoPE is then
    q' = R*C2 + rot(R)*S2 with C2=[cos;cos], S2=[-sin;sin] (3 full-width
    DVE ops + 2 ACT half-copies to build rot(R)).
  - 1/sqrt(head_dim) is folded into wq on the host.
  - Scores are computed transposed [ks, qs]; softmax needs no max
    subtraction (|S| <~ 12 for this data). Causal masking is one wide
    [128,4*512] multiply over the diagonal chunks. The denominator Z is a
    matmul with an all-ones stationary over DVE-pre-reduced quads, which
    also broadcasts Z across partitions for free; 1/Z uses the fast
    custom-DVE reciprocal. Normalization is folded into the PSUM->SBUF
    copy of the attention output.
  - Batch-0 attention groups interleave with batch-1 projection blocks so
    TensorE never drains; batch-1 groups run (0,3,2,1) so the tail is a
    small group.
"""

import math

import numpy as np
import ml_dtypes

BS, SEQ, DIM = 2, 2048, 2048
NH, NKV, HD = 16, 8, 128
S = BS * SEQ  # 4096
NCORES = 8
QH = NH // NCORES  # 2 q heads per core
MQ = QH * HD  # 256
SB = 512  # seq block
NSB = S // SB  # 8
NDC = DIM // 128  # 16 contraction chunks
QBLK = SEQ // SB  # 4 query blocks per batch
NKC_MAX = SEQ // 128  # 16

_CACHE = {}


def _build():
    import concourse.tile as tile
    from concourse import bacc, mybir

    BF = mybir.dt.bfloat16
    F32 = mybir.dt.float32
    Exp = mybir.ActivationFunctionType.Exp

    nc = bacc.Bacc(
        "TRN2", target_bir_lowering=False, debug=False, num_devices=NCORES
    )
    xR = nc.dram_tensor("xR", [128, NSB, NDC, SB], BF, kind="ExternalInput").ap()
    wqR = nc.dram_tensor("wqR", [128, NDC, MQ], BF, kind="ExternalInput").ap()
    wkR = nc.dram_tensor("wkR", [128, NDC, HD], BF, kind="ExternalInput").ap()
    wvR = nc.dram_tensor("wvR", [128, NDC, HD], BF, kind="ExternalInput").ap()
    woR = nc.dram_tensor("woR", [128, QH, DIM], BF, kind="ExternalInput").ap()
    cos2 = nc.dram_tensor("cos2", [128, SEQ], BF, kind="ExternalInput").ap()
    sin2 = nc.dram_tensor("sin2", [128, SEQ], BF, kind="ExternalInput").ap()
    maskw = nc.dram_tensor("maskw", [128, 4, SB], BF, kind="ExternalInput").ap()
    out = nc.dram_tensor("out", [S, DIM], BF, kind="ExternalOutput").ap()

    with tile.TileContext(nc, pool_alloc_mode="queue") as tc:
        with tc.tile_pool(name="pers", bufs=1) as pers, tc.tile_pool(
            name="w1", bufs=1
        ) as w1p, tc.tile_pool(name="xt", bufs=2) as xtp, tc.tile_pool(
            name="rt", bufs=3
        ) as rtp, tc.tile_pool(name="vt", bufs=2) as vtp, tc.tile_pool(
            name="st", bufs=2
        ) as stp, tc.tile_pool(name="zt", bufs=12) as ztp, tc.tile_pool(name="zr", bufs=2) as zrp, tc.tile_pool(
            name="os", bufs=6
        ) as osp, tc.tile_pool(name="pj", bufs=3, space="PSUM") as pjp, tc.tile_pool(
            name="ps", bufs=3, space="PSUM"
        ) as psp, tc.tile_pool(name="acc", bufs=2, space="PSUM") as psa:
            qt = pers.tile([128, QH, S], BF, tag="qt")  # Q^T per head [hd, s]
            kt = pers.tile([128, S], BF, tag="kt")  # K^T [hd, s]
            vsb = pers.tile([128, S // 128, HD], BF, tag="v")  # V [s, vd]
            at = pers.tile([128, QH, S], BF, tag="at")  # attnout^T [vd, s]
            wo_sb = pers.tile([128, QH, DIM], BF, tag="wo")
            cos_sb = pers.tile([128, SEQ], BF, tag="cos")
            sin_sb = pers.tile([128, SEQ], BF, tag="sin")
            mask_sb = pers.tile([128, 4, SB], BF, tag="mask")
            ones_sb = pers.tile([128, 128], BF, tag="ones")
            wq_sb = w1p.tile([128, NDC, MQ], BF, tag="wq")
            wk_sb = w1p.tile([128, NDC, HD], BF, tag="wk")
            wv_sb = w1p.tile([128, NDC, HD], BF, tag="wv")

            nc.vector.memset(ones_sb, 1.0)

            xt_tiles = {}

            def load_x(sb, split=False):
                # alternate x tiles between the two HWDGE queues
                eng = nc.sync if sb % 2 == 0 else nc.scalar
                t = xtp.tile([128, NDC, SB], BF, tag="xt")
                if split:
                    for c in range(4):
                        eng.dma_start(
                            t[:, 4 * c : 4 * c + 4, :],
                            xR[:, sb, 4 * c : 4 * c + 4, :],
                        )
                else:
                    eng.dma_start(t, xR[:, sb])
                xt_tiles[sb] = t

            # sync queue: wk then even x tiles (+ output writes later);
            # scalar queue: wq/wv, odd x tiles, rope tables, mask, wo.
            nc.sync.dma_start(wk_sb, wkR)
            load_x(0, split=True)
            nc.scalar.dma_start(wq_sb, wqR)
            nc.scalar.dma_start(wv_sb, wvR)
            load_x(1)
            nc.scalar.dma_start(cos_sb, cos2)
            nc.scalar.dma_start(sin_sb, sin2)
            nc.scalar.dma_start(mask_sb, maskw)
            nc.scalar.dma_start(wo_sb, woR)

            def phase1(sb):
                xt_t = xt_tiles.pop(sb)
                s0 = sb * SB
                seq0 = (sb % QBLK) * SB
                cs = cos_sb[:, seq0 : seq0 + SB]
                sn = sin_sb[:, seq0 : seq0 + SB]
                # K first (feeds scores soonest), then Q heads
                for which in (QH, 0, 1):
                    pst = pjp.tile([128, SB], F32, tag="pj")
                    for dc in range(NDC):
                        if which < QH:
                            lhs = wq_sb[:, dc, which * 128 : (which + 1) * 128]
                        else:
                            lhs = wk_sb[:, dc, :]
                        nc.tensor.matmul(
                            pst,
                            lhs,
                            xt_t[:, dc, :],
                            start=(dc == 0),
                            stop=(dc == NDC - 1),
                        )
                    if which < QH:
                        dest = qt[:, which, s0 : s0 + SB]
                    else:
                        dest = kt[:, s0 : s0 + SB]
                    rot = rtp.tile([128, SB], BF, tag="rot")
                    nc.scalar.copy(rot[64:128, :], pst[0:64, :])
                    nc.scalar.copy(rot[0:64, :], pst[64:128, :])
                    t1 = rtp.tile([128, SB], BF, tag="t1")
                    nc.vector.tensor_mul(t1, pst, cs)
                    t2 = rtp.tile([128, SB], BF, tag="t2")
                    nc.vector.tensor_mul(t2, rot, sn)
                    nc.vector.tensor_add(dest, t1, t2)
                # V^T [vd, s] with N=512 matmuls, then DMA-transpose back to
                # natural [s, vd] (2 transposes per HWDGE queue)
                psv = pjp.tile([128, SB], F32, tag="pj")
                for dc in range(NDC):
                    nc.tensor.matmul(
                        psv,
                        wv_sb[:, dc, :],
                        xt_t[:, dc, :],
                        start=(dc == 0),
                        stop=(dc == NDC - 1),
                    )
                vtt = vtp.tile([128, SB], BF, tag="vt")
                nc.vector.tensor_copy(vtt, psv)
                for c in range(SB // 128):
                    eng = nc.sync if c % 2 == 0 else nc.scalar
                    eng.dma_start_transpose(
                        vsb[:, sb * 4 + c, :], vtt[:, c * 128 : (c + 1) * 128]
                    )

            def kc_order(qb, nkc):
                return list(range(4 * qb, nkc)) + list(range(4 * qb))

            def part1(b, qb, h):
                """Scores + exp + wide mask + quad tree."""
                nkc = 4 * (qb + 1)
                qs0 = b * SEQ + qb * SB
                order = kc_order(qb, nkc)
                st_t = stp.tile([128, NKC_MAX, SB], BF, tag="st")
                for kc in order:
                    st_ps = psp.tile([128, SB], F32, tag="ps")
                    nc.tensor.matmul(
                        st_ps,
                        kt[:, b * SEQ + kc * 128 : b * SEQ + (kc + 1) * 128],
                        qt[:, h, qs0 : qs0 + SB],
                        start=True,
                        stop=True,
                    )
                    nc.scalar.activation(st_t[:, kc, :], st_ps, Exp)
                # causal mask over the 4 diagonal chunks in one wide op
                dg = 4 * qb
                nc.vector.tensor_mul(
                    st_t[:, dg : dg + 4, :], st_t[:, dg : dg + 4, :], mask_sb
                )
                # quad pre-reduction (each quad is 4 consecutive chunks),
                # then reduce quads to a single tile so Z is one matmul
                quads = []
                for qi in range(nkc // 4):
                    c0 = order[4 * qi]
                    p0 = ztp.tile([128, SB], BF, tag="zt")
                    nc.vector.tensor_add(p0, st_t[:, c0, :], st_t[:, c0 + 1, :])
                    p1 = ztp.tile([128, SB], BF, tag="zt")
                    nc.vector.tensor_add(
                        p1, st_t[:, c0 + 2, :], st_t[:, c0 + 3, :]
                    )
                    q0 = ztp.tile([128, SB], BF, tag="zt")
                    nc.vector.tensor_add(q0, p0, p1)
                    quads.append(q0)
                while len(quads) > 1:
                    nxt = []
                    for i in range(0, len(quads) - 1, 2):
                        s = ztp.tile([128, SB], BF, tag="zt")
                        nc.vector.tensor_add(s, quads[i], quads[i + 1])
                        nxt.append(s)
                    if len(quads) % 2:
                        nxt.append(quads[-1])
                    quads = nxt
                return st_t, quads

            def part2(b, qb, h, st_t, quads):
                """Z matmul, fast reciprocal, PV, at-scale for one group."""
                nkc = 4 * (qb + 1)
                qs0 = b * SEQ + qb * SB
                order = kc_order(qb, nkc)
                z_ps = psa.tile([128, SB], F32, tag="acc")
                o_ps = psa.tile([128, SB], F32, tag="acc")
                for i, q0 in enumerate(quads):
                    nc.tensor.matmul(
                        z_ps,
                        ones_sb,
                        q0,
                        start=(i == 0),
                        stop=(i == len(quads) - 1),
                    )
                zr_t = zrp.tile([128, SB], F32, tag="zr")
                nc.vector.reciprocal_approx_fast(zr_t, z_ps)
                for i, kc in enumerate(order):
                    nc.tensor.matmul(
                        o_ps,
                        vsb[:, b * (SEQ // 128) + kc, :],
                        st_t[:, kc, :],
                        start=(i == 0),
                        stop=(i == nkc - 1),
                    )
                nc.vector.tensor_mul(at[:, h, qs0 : qs0 + SB], o_ps, zr_t)

            def emit_wo(b, qb):
                for gcl in range(SB // 128):
                    gc = (b * SEQ + qb * SB) // 128 + gcl
                    for ob in range(DIM // SB):
                        op_ps = pjp.tile([128, SB], F32, tag="pj")
                        for jc in range(QH):
                            nc.tensor.matmul(
                                op_ps,
                                at[:, jc, gc * 128 : (gc + 1) * 128],
                                wo_sb[:, jc, ob * SB : (ob + 1) * SB],
                                start=(jc == 0),
                                stop=(jc == QH - 1),
                            )
                        ot = osp.tile([128, SB], BF, tag="os")
                        if ob % 2:
                            nc.scalar.copy(ot, op_ps)
                        else:
                            nc.vector.tensor_copy(ot, op_ps)
                        nc.sync.dma_start(
                            out[
                                gc * 128 : (gc + 1) * 128,
                                ob * SB : (ob + 1) * SB,
                            ],
                            ot,
                        )

            # ---- interleaved schedule ----
            P1 = "p1"
            seq_items = [
                (P1, 0),
                (0, 0, 0),
                (P1, 1),
                (0, 0, 1),
                (P1, 2),
                (0, 1, 0),
                (P1, 3),
                (0, 1, 1),
                (P1, 4),
                (0, 2, 0),
                (P1, 5),
                (0, 2, 1),
                (P1, 6),
                (0, 3, 0),
                (P1, 7),
                (0, 3, 1),
                (1, 3, 0),
                (1, 3, 1),
                (1, 2, 0),
                (1, 2, 1),
                (1, 1, 0),
                (1, 1, 1),
                (1, 0, 0),
                (1, 0, 1),
            ]

            prev = None
            for item in seq_items:
                if item[0] == P1:
                    sb = item[1]
                    phase1(sb)
                    if sb + 2 < NSB:
                        load_x(sb + 2)
                    continue
                b, qb, h = item
                st_t, quads = part1(b, qb, h)
                if prev is not None:
                    pb, pqb, ph, pst, pqs = prev
                    part2(pb, pqb, ph, pst, pqs)
                    if ph == QH - 1:
                        emit_wo(pb, pqb)
                prev = (b, qb, h, st_t, quads)
            pb, pqb, ph, pst, pqs = prev
            part2(pb, pqb, ph, pst, pqs)
            emit_wo(pb, pqb)

    nc.compile()
    return nc


def _prep_inputs(x, freqs_cos, freqs_sin, wq, wk, wv, wo):
    bf16 = ml_dtypes.bfloat16
    x2 = np.asarray(x, dtype=np.float32).reshape(S, DIM).T  # [DIM, S]
    # xR[p, sb, dc, s] = x2[dc*128+p, sb*SB+s]
    xR = np.ascontiguousarray(
        x2.reshape(NDC, 128, NSB, SB).transpose(1, 2, 0, 3)
    ).astype(bf16)
    cos = np.asarray(freqs_cos, np.float32).T  # [64, SEQ]
    sin = np.asarray(freqs_sin, np.float32).T
    cos2 = np.ascontiguousarray(np.concatenate([cos, cos], axis=0)).astype(bf16)
    sin2 = np.ascontiguousarray(np.concatenate([-sin, sin], axis=0)).astype(bf16)
    # maskw[p, r, q] = 1 iff q >= p + r*128
    pidx = np.arange(128)[:, None, None]
    ridx = np.arange(4)[None, :, None]
    qidx = np.arange(SB)[None, None, :]
    maskw = (qidx >= pidx + ridx * 128).astype(bf16)
    perm = np.concatenate([np.arange(0, HD, 2), np.arange(1, HD, 2)])
    scale = 1.0 / math.sqrt(HD)
    wq = np.asarray(wq, np.float32)
    wk = np.asarray(wk, np.float32)
    wv = np.asarray(wv, np.float32)
    wo = np.asarray(wo, np.float32)

    def part_major(wT, ncols):
        # wT: [DIM, ncols] -> [128, NDC, ncols]
        return np.ascontiguousarray(
            wT.reshape(NDC, 128, ncols).transpose(1, 0, 2)
        ).astype(bf16)

    in_maps = []
    for i in range(NCORES):
        wq_i = (wq[i * MQ : (i + 1) * MQ] * scale).reshape(QH, HD, DIM)[
            :, perm, :
        ].reshape(MQ, DIM)
        wk_i = wk[i * HD : (i + 1) * HD][perm]
        wv_i = wv[i * HD : (i + 1) * HD]
        wo_i = wo[:, i * MQ : (i + 1) * MQ]  # [DIM, MQ]
        woT = wo_i.T  # [MQ, DIM]
        woR = np.ascontiguousarray(
            woT.reshape(QH, 128, DIM).transpose(1, 0, 2)
        ).astype(bf16)
        in_maps.append(
            {
                "xR": xR,
                "wqR": part_major(np.ascontiguousarray(wq_i.T), MQ),
                "wkR": part_major(np.ascontiguousarray(wk_i.T), HD),
                "wvR": part_major(np.ascontiguousarray(wv_i.T), HD),
                "woR": woR,
                "cos2": cos2,
                "sin2": sin2,
                "maskw": maskw,
            }
        )
    return in_maps


def _run(inputs, trace=False):
    from concourse.bass_utils import run_bass_kernel_spmd

    if "nc" not in _CACHE:
        _CACHE["nc"] = _build()
    nc = _CACHE["nc"]
    in_maps = _prep_inputs(**inputs)
    res = run_bass_kernel_spmd(
        nc, in_maps, core_ids=list(range(NCORES)), trace=trace
    )
    partials = [np.asarray(r["out"], np.float32) for r in res.results]
    full = np.sum(partials, axis=0).reshape(BS, SEQ, DIM).astype(np.float32)
    return full, res


def kernel(**inputs):
    full, _ = _run(inputs, trace=False)
    return full


# revision 12
# speedup vs baseline: 1.1157x; 1.1157x over previous
"""Head-parallel GQA attention kernel for 8 TRN2 NeuronCores.

Sharding: core i owns KV head i and Q heads (2i, 2i+1), plus the matching
256-column slice of wo's input dim. Each core computes a partial output
(its heads' contribution through wo); the host sums the 8 partials.

All device compute is bf16 (PSUM accumulation in f32). The host pre-bakes
layouts so the device never rearranges inputs:
  - All DRAM inputs are partition-major [128, ...] with contiguous
    per-partition lines, so every load is ~128 fat descriptors.
  - wq/wk rows are permuted per-head (evens then odds) so interleaved-pair
    RoPE becomes rotate-half form: pairs live in partition halves [0:64]
    and [64:128] of the projected Q^T/K^T tiles. RoPE is then
    q' = R*C2 + rot(R)*S2 with C2=[cos;cos], S2=[-sin;sin] (3 full-width
    DVE ops + 2 ACT half-copies to build rot(R)).
  - 1/sqrt(head_dim) is folded into wq on the host.
  - Scores are computed transposed [ks, qs]; softmax needs no max
    subtraction (|S| <~ 12 for this data). Causal masking is one wide
    [128,4*512] multiply over the diagonal chunks. The denominator Z is a
    matmul with an all-ones stationary over DVE-pre-reduced quads, which
    also broadcasts Z across partitions for free; 1/Z uses the fast
    custom-DVE reciprocal. Normalization is folded into the PSUM->SBUF
    copy of the attention output.
  - Batch-0 attention groups interleave with batch-1 projection blocks so
    TensorE never drains; batch-1 groups run (0,3,2,1) so the tail is a
    small group.
"""

import math

import numpy as np
import ml_dtypes

BS, SEQ, DIM = 2, 2048, 2048
NH, NKV, HD = 16, 8, 128
S = BS * SEQ  # 4096
NCORES = 8
QH = NH // NCORES  # 2 q heads per core
MQ = QH * HD  # 256
SB = 512  # seq block
NSB = S // SB  # 8
NDC = DIM // 128  # 16 contraction chunks
QBLK = SEQ // SB  # 4 query blocks per batch
NKC_MAX = SEQ // 128  # 16

_CACHE = {}


def _build():
    import concourse.tile as tile
    from concourse import bacc, mybir

    BF = mybir.dt.bfloat16
    F32 = mybir.dt.float32
    Exp = mybir.ActivationFunctionType.Exp

    nc = bacc.Bacc(
        "TRN2", target_bir_lowering=False, debug=False, num_devices=NCORES
    )
    xR = nc.dram_tensor("xR", [128, NSB, NDC, SB], BF, kind="ExternalInput").ap()
    wqR = nc.dram_tensor("wqR", [128, NDC, MQ], BF, kind="ExternalInput").ap()
    wkR = nc.dram_tensor("wkR", [128, NDC, HD], BF, kind="ExternalInput").ap()
    wvR = nc.dram_tensor("wvR", [128, NDC, HD], BF, kind="ExternalInput").ap()
    woR = nc.dram_tensor("woR", [128, QH, DIM], BF, kind="ExternalInput").ap()
    cos2 = nc.dram_tensor("cos2", [128, SEQ], BF, kind="ExternalInput").ap()
    sin2 = nc.dram_tensor("sin2", [128, SEQ], BF, kind="ExternalInput").ap()
    maskw = nc.dram_tensor("maskw", [128, 4, SB], BF, kind="ExternalInput").ap()
    out = nc.dram_tensor("out", [S, DIM], BF, kind="ExternalOutput").ap()

    with tile.TileContext(nc, pool_alloc_mode="queue") as tc:
        with tc.tile_pool(name="pers", bufs=1) as pers, tc.tile_pool(
            name="w1", bufs=1
        ) as w1p, tc.tile_pool(name="xt", bufs=2) as xtp, tc.tile_pool(
            name="rt", bufs=3
        ) as rtp, tc.tile_pool(name="vt", bufs=2) as vtp, tc.tile_pool(
            name="st", bufs=2
        ) as stp, tc.tile_pool(name="zt", bufs=12) as ztp, tc.tile_pool(name="zr", bufs=2) as zrp, tc.tile_pool(
            name="os", bufs=6
        ) as osp, tc.tile_pool(name="pj", bufs=3, space="PSUM") as pjp, tc.tile_pool(
            name="ps", bufs=3, space="PSUM"
        ) as psp, tc.tile_pool(name="acc", bufs=2, space="PSUM") as psa:
            qt = pers.tile([128, QH, S], BF, tag="qt")  # Q^T per head [hd, s]
            kt = pers.tile([128, S], BF, tag="kt")  # K^T [hd, s]
            vsb = pers.tile([128, S // 128, HD], BF, tag="v")  # V [s, vd]
            at = pers.tile([128, QH, S], BF, tag="at")  # attnout^T [vd, s]
            wo_sb = pers.tile([128, QH, DIM], BF, tag="wo")
            cos_sb = pers.tile([128, SEQ], BF, tag="cos")
            sin_sb = pers.tile([128, SEQ], BF, tag="sin")
            mask_sb = pers.tile([128, 4, SB], BF, tag="mask")
            ones_sb = pers.tile([128, 128], BF, tag="ones")
            wq_sb = w1p.tile([128, NDC, MQ], BF, tag="wq")
            wk_sb = w1p.tile([128, NDC, HD], BF, tag="wk")
            wv_sb = w1p.tile([128, NDC, HD], BF, tag="wv")

            nc.vector.memset(ones_sb, 1.0)

            xt_tiles = {}

            def load_x(sb, split=False):
                # alternate x tiles between the two HWDGE queues
                eng = nc.sync if sb % 2 == 0 else nc.scalar
                t = xtp.tile([128, NDC, SB], BF, tag="xt")
                if split:
                    for c in range(4):
                        eng.dma_start(
                            t[:, 4 * c : 4 * c + 4, :],
                            xR[:, sb, 4 * c : 4 * c + 4, :],
                        )
                else:
                    eng.dma_start(t, xR[:, sb])
                xt_tiles[sb] = t

            # sync queue: wk then even x tiles (+ output writes later);
            # scalar queue: wq/wv, odd x tiles, rope tables, mask, wo.
            nc.sync.dma_start(wk_sb, wkR)
            load_x(0, split=True)
            nc.scalar.dma_start(wq_sb, wqR)
            nc.scalar.dma_start(wv_sb, wvR)
            load_x(1)
            nc.scalar.dma_start(cos_sb, cos2)
            nc.scalar.dma_start(sin_sb, sin2)
            nc.scalar.dma_start(mask_sb, maskw)
            nc.scalar.dma_start(wo_sb, woR)

            def phase1(sb):
                xt_t = xt_tiles.pop(sb)
                s0 = sb * SB
                seq0 = (sb % QBLK) * SB
                cs = cos_sb[:, seq0 : seq0 + SB]
                sn = sin_sb[:, seq0 : seq0 + SB]
                # K first (feeds scores soonest), then Q heads
                for which in (QH, 0, 1):
                    pst = pjp.tile([128, SB], F32, tag="pj")
                    for dc in range(NDC):
                        if which < QH:
                            lhs = wq_sb[:, dc, which * 128 : (which + 1) * 128]
                        else:
                            lhs = wk_sb[:, dc, :]
                        nc.tensor.matmul(
                            pst,
                            lhs,
                            xt_t[:, dc, :],
                            start=(dc == 0),
                            stop=(dc == NDC - 1),
                        )
                    if which < QH:
                        dest = qt[:, which, s0 : s0 + SB]
                    else:
                        dest = kt[:, s0 : s0 + SB]
                    rot = rtp.tile([128, SB], BF, tag="rot")
                    nc.scalar.copy(rot[64:128, :], pst[0:64, :])
                    nc.scalar.copy(rot[0:64, :], pst[64:128, :])
                    t1 = rtp.tile([128, SB], BF, tag="t1")
                    nc.vector.tensor_mul(t1, pst, cs)
                    t2 = rtp.tile([128, SB], BF, tag="t2")
                    nc.vector.tensor_mul(t2, rot, sn)
                    nc.vector.tensor_add(dest, t1, t2)
                # V natural [s, vd]
                for sc in range(SB // 128):
                    psv = pjp.tile([128, SB], F32, tag="pj")
                    pv0 = psv[:, 0:HD]
                    for dc in range(NDC):
                        nc.tensor.matmul(
                            pv0,
                            xt_t[:, dc, sc * 128 : (sc + 1) * 128],
                            wv_sb[:, dc, :],
                            start=(dc == 0),
                            stop=(dc == NDC - 1),
                        )
                    nc.vector.tensor_copy(vsb[:, sb * 4 + sc, :], pv0)

            def kc_order(qb, nkc):
                return list(range(4 * qb, nkc)) + list(range(4 * qb))

            def part1(b, qb, h):
                """Scores + exp + wide mask + quad tree."""
                nkc = 4 * (qb + 1)
                qs0 = b * SEQ + qb * SB
                order = kc_order(qb, nkc)
                st_t = stp.tile([128, NKC_MAX, SB], BF, tag="st")
                for kc in order:
                    st_ps = psp.tile([128, SB], F32, tag="ps")
                    nc.tensor.matmul(
                        st_ps,
                        kt[:, b * SEQ + kc * 128 : b * SEQ + (kc + 1) * 128],
                        qt[:, h, qs0 : qs0 + SB],
                        start=True,
                        stop=True,
                    )
                    nc.scalar.activation(st_t[:, kc, :], st_ps, Exp)
                # causal mask over the 4 diagonal chunks in one wide op
                dg = 4 * qb
                nc.vector.tensor_mul(
                    st_t[:, dg : dg + 4, :], st_t[:, dg : dg + 4, :], mask_sb
                )
                # quad pre-reduction (each quad is 4 consecutive chunks),
                # then reduce quads to a single tile so Z is one matmul
                quads = []
                for qi in range(nkc // 4):
                    c0 = order[4 * qi]
                    p0 = ztp.tile([128, SB], BF, tag="zt")
                    nc.vector.tensor_add(p0, st_t[:, c0, :], st_t[:, c0 + 1, :])
                    p1 = ztp.tile([128, SB], BF, tag="zt")
                    nc.vector.tensor_add(
                        p1, st_t[:, c0 + 2, :], st_t[:, c0 + 3, :]
                    )
                    q0 = ztp.tile([128, SB], BF, tag="zt")
                    nc.vector.tensor_add(q0, p0, p1)
                    quads.append(q0)
                while len(quads) > 1:
                    nxt = []
                    for i in range(0, len(quads) - 1, 2):
                        s = ztp.tile([128, SB], BF, tag="zt")
                        nc.vector.tensor_add(s, quads[i], quads[i + 1])
                        nxt.append(s)
                    if len(quads) % 2:
                        nxt.append(quads[-1])
                    quads = nxt
                return st_t, quads

            def part2(b, qb, h, st_t, quads):
                """Z matmul, fast reciprocal, PV, at-scale for one group."""
                nkc = 4 * (qb + 1)
                qs0 = b * SEQ + qb * SB
                order = kc_order(qb, nkc)
                z_ps = psa.tile([128, SB], F32, tag="acc")
                o_ps = psa.tile([128, SB], F32, tag="acc")
                for i, q0 in enumerate(quads):
                    nc.tensor.matmul(
                        z_ps,
                        ones_sb,
                        q0,
                        start=(i == 0),
                        stop=(i == len(quads) - 1),
                    )
                zr_t = zrp.tile([128, SB], F32, tag="zr")
                nc.vector.reciprocal_approx_fast(zr_t, z_ps)
                for i, kc in enumerate(order):
                    nc.tensor.matmul(
                        o_ps,
                        vsb[:, b * (SEQ // 128) + kc, :],
                        st_t[:, kc, :],
                        start=(i == 0),
                        stop=(i == nkc - 1),
                    )
                nc.vector.tensor_mul(at[:, h, qs0 : qs0 + SB], o_ps, zr_t)

            def emit_wo(b, qb):
                for gcl in range(SB // 128):
                    gc = (b * SEQ + qb * SB) // 128 + gcl
                    for ob in range(DIM // SB):
                        op_ps = pjp.tile([128, SB], F32, tag="pj")
                        for jc in range(QH):
                            nc.tensor.matmul(
                                op_ps,
                                at[:, jc, gc * 128 : (gc + 1) * 128],
                                wo_sb[:, jc, ob * SB : (ob + 1) * SB],
                                start=(jc == 0),
                                stop=(jc == QH - 1),
                            )
                        ot = osp.tile([128, SB], BF, tag="os")
                        if ob % 2:
                            nc.scalar.copy(ot, op_ps)
                        else:
                            nc.vector.tensor_copy(ot, op_ps)
                        nc.sync.dma_start(
                            out[
                                gc * 128 : (gc + 1) * 128,
                                ob * SB : (ob + 1) * SB,
                            ],
                            ot,
                        )

            # ---- interleaved schedule ----
            P1 = "p1"
            seq_items = [
                (P1, 0),
                (0, 0, 0),
                (P1, 1),
                (0, 0, 1),
                (P1, 2),
                (0, 1, 0),
                (P1, 3),
                (0, 1, 1),
                (P1, 4),
                (0, 2, 0),
                (P1, 5),
                (0, 2, 1),
                (P1, 6),
                (0, 3, 0),
                (P1, 7),
                (0, 3, 1),
                (1, 3, 0),
                (1, 3, 1),
                (1, 2, 0),
                (1, 2, 1),
                (1, 1, 0),
                (1, 1, 1),
                (1, 0, 0),
                (1, 0, 1),
            ]

            prev = None
            for item in seq_items:
                if item[0] == P1:
                    sb = item[1]
                    phase1(sb)
                    if sb + 2 < NSB:
                        load_x(sb + 2)
                    continue
                b, qb, h = item
                st_t, quads = part1(b, qb, h)
                if prev is not None:
                    pb, pqb, ph, pst, pqs = prev
                    part2(pb, pqb, ph, pst, pqs)
                    if ph == QH - 1:
                        emit_wo(pb, pqb)
                prev = (b, qb, h, st_t, quads)
            pb, pqb, ph, pst, pqs = prev
            part2(pb, pqb, ph, pst, pqs)
            emit_wo(pb, pqb)

    nc.compile()
    return nc


def _prep_inputs(x, freqs_cos, freqs_sin, wq, wk, wv, wo):
    bf16 = ml_dtypes.bfloat16
    x2 = np.asarray(x, dtype=np.float32).reshape(S, DIM).T  # [DIM, S]
    # xR[p, sb, dc, s] = x2[dc*128+p, sb*SB+s]
    xR = np.ascontiguousarray(
        x2.reshape(NDC, 128, NSB, SB).transpose(1, 2, 0, 3)
    ).astype(bf16)
    cos = np.asarray(freqs_cos, np.float32).T  # [64, SEQ]
    sin = np.asarray(freqs_sin, np.float32).T
    cos2 = np.ascontiguousarray(np.concatenate([cos, cos], axis=0)).astype(bf16)
    sin2 = np.ascontiguousarray(np.concatenate([-sin, sin], axis=0)).astype(bf16)
    # maskw[p, r, q] = 1 iff q >= p + r*128
    pidx = np.arange(128)[:, None, None]
    ridx = np.arange(4)[None, :, None]
    qidx = np.arange(SB)[None, None, :]
    maskw = (qidx >= pidx + ridx * 128).astype(bf16)
    perm = np.concatenate([np.arange(0, HD, 2), np.arange(1, HD, 2)])
    scale = 1.0 / math.sqrt(HD)
    wq = np.asarray(wq, np.float32)
    wk = np.asarray(wk, np.float32)
    wv = np.asarray(wv, np.float32)
    wo = np.asarray(wo, np.float32)

    def part_major(wT, ncols):
        # wT: [DIM, ncols] -> [128, NDC, ncols]
        return np.ascontiguousarray(
            wT.reshape(NDC, 128, ncols).transpose(1, 0, 2)
        ).astype(bf16)

    in_maps = []
    for i in range(NCORES):
        wq_i = (wq[i * MQ : (i + 1) * MQ] * scale).reshape(QH, HD, DIM)[
            :, perm, :
        ].reshape(MQ, DIM)
        wk_i = wk[i * HD : (i + 1) * HD][perm]
        wv_i = wv[i * HD : (i + 1) * HD]
        wo_i = wo[:, i * MQ : (i + 1) * MQ]  # [DIM, MQ]
        woT = wo_i.T  # [MQ, DIM]
        woR = np.ascontiguousarray(
            woT.reshape(QH, 128, DIM).transpose(1, 0, 2)
        ).astype(bf16)
        in_maps.append(
            {
                "xR": xR,
                "wqR": part_major(np.ascontiguousarray(wq_i.T), MQ),
                "wkR": part_major(np.ascontiguousarray(wk_i.T), HD),
                "wvR": part_major(np.ascontiguousarray(wv_i.T), HD),
                "woR": woR,
                "cos2": cos2,
                "sin2": sin2,
                "maskw": maskw,
            }
        )
    return in_maps


def _run(inputs, trace=False):
    from concourse.bass_utils import run_bass_kernel_spmd

    if "nc" not in _CACHE:
        _CACHE["nc"] = _build()
    nc = _CACHE["nc"]
    in_maps = _prep_inputs(**inputs)
    res = run_bass_kernel_spmd(
        nc, in_maps, core_ids=list(range(NCORES)), trace=trace
    )
    partials = [np.asarray(r["out"], np.float32) for r in res.results]
    full = np.sum(partials, axis=0).reshape(BS, SEQ, DIM).astype(np.float32)
    return full, res


def kernel(**inputs):
    full, _ = _run(inputs, trace=False)
    return full


# revision 16
# speedup vs baseline: 1.1778x; 1.0557x over previous
"""Head-parallel GQA attention kernel for 8 TRN2 NeuronCores.

Sharding: core i owns KV head i and Q heads (2i, 2i+1), plus the matching
256-column slice of wo's input dim. Each core computes a partial output
(its heads' contribution through wo); the host sums the 8 partials.

All device compute is bf16 (PSUM accumulation in f32). The host pre-bakes
layouts so the device never rearranges inputs:
  - All DRAM inputs are partition-major [128, ...] with contiguous
    per-partition lines, so every load is ~128 fat descriptors.
  - wq/wk rows are permuted per-head (evens then odds) so interleaved-pair
    RoPE becomes rotate-half form: pairs live in partition halves [0:64]
    and [64:128] of the projected Q^T/K^T tiles. RoPE is then
    q' = R*C2 + rot(R)*S2 with C2=[cos;cos], S2=[-sin;sin] (3 full-width
    DVE ops + 2 ACT half-copies to build rot(R)).
  - 1/sqrt(head_dim) is folded into wq on the host.
  - Scores are computed transposed [ks, qs]; softmax needs no max
    subtraction (|S| <~ 12 for this data). Causal masking is one wide
    [128,4*512] multiply over the diagonal chunks. The denominator Z is a
    matmul with an all-ones stationary over DVE-pre-reduced quads, which
    also broadcasts Z across partitions for free; 1/Z uses the fast
    custom-DVE reciprocal. Normalization is folded into the PSUM->SBUF
    copy of the attention output.
  - Batch-0 attention groups interleave with batch-1 projection blocks so
    TensorE never drains; batch-1 groups run (0,3,2,1) so the tail is a
    small group.
"""

import math

import numpy as np
import ml_dtypes

BS, SEQ, DIM = 2, 2048, 2048
NH, NKV, HD = 16, 8, 128
S = BS * SEQ  # 4096
NCORES = 8
QH = NH // NCORES  # 2 q heads per core
MQ = QH * HD  # 256
SB = 512  # seq block
NSB = S // SB  # 8
NDC = DIM // 128  # 16 contraction chunks
QBLK = SEQ // SB  # 4 query blocks per batch
NKC_MAX = SEQ // 128  # 16

_CACHE = {}


def _build():
    import concourse.tile as tile
    from concourse import bacc, mybir

    BF = mybir.dt.bfloat16
    F32 = mybir.dt.float32
    Exp = mybir.ActivationFunctionType.Exp

    nc = bacc.Bacc(
        "TRN2", target_bir_lowering=False, debug=False, num_devices=NCORES
    )
    xR = nc.dram_tensor("xR", [128, NSB, NDC, SB], BF, kind="ExternalInput").ap()
    wqR = nc.dram_tensor("wqR", [128, NDC, MQ], BF, kind="ExternalInput").ap()
    wkR = nc.dram_tensor("wkR", [128, NDC, HD], BF, kind="ExternalInput").ap()
    wvR = nc.dram_tensor("wvR", [128, NDC, HD], BF, kind="ExternalInput").ap()
    woR = nc.dram_tensor("woR", [128, QH, DIM], BF, kind="ExternalInput").ap()
    cos2 = nc.dram_tensor("cos2", [128, SEQ], BF, kind="ExternalInput").ap()
    sin2 = nc.dram_tensor("sin2", [128, SEQ], BF, kind="ExternalInput").ap()
    maskw = nc.dram_tensor("maskw", [128, 4, SB], BF, kind="ExternalInput").ap()
    out = nc.dram_tensor("out", [S, DIM], BF, kind="ExternalOutput").ap()

    with tile.TileContext(nc, pool_alloc_mode="queue") as tc:
        with tc.tile_pool(name="pers", bufs=1) as pers, tc.tile_pool(
            name="w1", bufs=1
        ) as w1p, tc.tile_pool(name="xt", bufs=2) as xtp, tc.tile_pool(
            name="rt", bufs=3
        ) as rtp, tc.tile_pool(name="vt", bufs=2) as vtp, tc.tile_pool(
            name="st", bufs=2
        ) as stp, tc.tile_pool(name="zt", bufs=12) as ztp, tc.tile_pool(name="zr", bufs=2) as zrp, tc.tile_pool(
            name="os", bufs=6
        ) as osp, tc.tile_pool(name="pj", bufs=3, space="PSUM") as pjp, tc.tile_pool(
            name="ps", bufs=3, space="PSUM"
        ) as psp, tc.tile_pool(name="acc", bufs=2, space="PSUM") as psa:
            qt = pers.tile([128, QH, S], BF, tag="qt")  # Q^T per head [hd, s]
            kt = pers.tile([128, S], BF, tag="kt")  # K^T [hd, s]
            vsb = pers.tile([128, S // 128, HD], BF, tag="v")  # V [s, vd]
            at = pers.tile([128, QH, S], BF, tag="at")  # attnout^T [vd, s]
            wo_sb = pers.tile([128, QH, DIM], BF, tag="wo")
            cos_sb = pers.tile([128, SEQ], BF, tag="cos")
            sin_sb = pers.tile([128, SEQ], BF, tag="sin")
            mask_sb = pers.tile([128, 4, SB], BF, tag="mask")
            ones_sb = pers.tile([128, 128], BF, tag="ones")
            wq_sb = w1p.tile([128, NDC, MQ], BF, tag="wq")
            wk_sb = w1p.tile([128, NDC, HD], BF, tag="wk")
            wv_sb = w1p.tile([128, NDC, HD], BF, tag="wv")

            nc.vector.memset(ones_sb, 1.0)

            xt_tiles = {}

            def load_x(sb, split=False):
                t = xtp.tile([128, NDC, SB], BF, tag="xt")
                if split:
                    nc.sync.dma_start(t[:, 0:8, :], xR[:, sb, 0:8, :])
                    nc.sync.dma_start(t[:, 8:NDC, :], xR[:, sb, 8:NDC, :])
                else:
                    nc.sync.dma_start(t, xR[:, sb])
                xt_tiles[sb] = t

            # sync queue: wk then x stream; scalar queue: the rest.
            nc.sync.dma_start(wk_sb, wkR)
            load_x(0, split=True)
            nc.scalar.dma_start(wq_sb, wqR)
            nc.scalar.dma_start(wv_sb, wvR)
            load_x(1)
            nc.scalar.dma_start(cos_sb, cos2)
            nc.scalar.dma_start(sin_sb, sin2)
            nc.scalar.dma_start(mask_sb, maskw)
            nc.scalar.dma_start(wo_sb, woR)

            def phase1(sb):
                xt_t = xt_tiles.pop(sb)
                s0 = sb * SB
                seq0 = (sb % QBLK) * SB
                cs = cos_sb[:, seq0 : seq0 + SB]
                sn = sin_sb[:, seq0 : seq0 + SB]
                # K first (feeds scores soonest), then Q heads
                for which in (QH, 0, 1):
                    pst = pjp.tile([128, SB], F32, tag="pj")
                    for dc in range(NDC):
                        if which < QH:
                            lhs = wq_sb[:, dc, which * 128 : (which + 1) * 128]
                        else:
                            lhs = wk_sb[:, dc, :]
                        nc.tensor.matmul(
                            pst,
                            lhs,
                            xt_t[:, dc, :],
                            start=(dc == 0),
                            stop=(dc == NDC - 1),
                        )
                    if which < QH:
                        dest = qt[:, which, s0 : s0 + SB]
                    else:
                        dest = kt[:, s0 : s0 + SB]
                    rot = rtp.tile([128, SB], BF, tag="rot")
                    nc.scalar.copy(rot[64:128, :], pst[0:64, :])
                    nc.scalar.copy(rot[0:64, :], pst[64:128, :])
                    t1 = rtp.tile([128, SB], BF, tag="t1")
                    nc.vector.tensor_mul(t1, pst, cs)
                    t2 = rtp.tile([128, SB], BF, tag="t2")
                    nc.vector.tensor_mul(t2, rot, sn)
                    nc.vector.tensor_add(dest, t1, t2)
                # V natural [s, vd]
                for sc in range(SB // 128):
                    psv = pjp.tile([128, SB], F32, tag="pj")
                    pv0 = psv[:, 0:HD]
                    for dc in range(NDC):
                        nc.tensor.matmul(
                            pv0,
                            xt_t[:, dc, sc * 128 : (sc + 1) * 128],
                            wv_sb[:, dc, :],
                            start=(dc == 0),
                            stop=(dc == NDC - 1),
                        )
                    nc.vector.tensor_copy(vsb[:, sb * 4 + sc, :], pv0)

            def kc_order(qb, nkc):
                return list(range(4 * qb, nkc)) + list(range(4 * qb))

            def part1(b, qb, h):
                """Scores + exp + wide mask + quad tree."""
                nkc = 4 * (qb + 1)
                qs0 = b * SEQ + qb * SB
                order = kc_order(qb, nkc)
                st_t = stp.tile([128, NKC_MAX, SB], BF, tag="st")
                for kc in order:
                    st_ps = psp.tile([128, SB], F32, tag="ps")
                    nc.tensor.matmul(
                        st_ps,
                        kt[:, b * SEQ + kc * 128 : b * SEQ + (kc + 1) * 128],
                        qt[:, h, qs0 : qs0 + SB],
                        start=True,
                        stop=True,
                    )
                    nc.scalar.activation(st_t[:, kc, :], st_ps, Exp)
                # causal mask over the 4 diagonal chunks in one wide op
                dg = 4 * qb
                nc.vector.tensor_mul(
                    st_t[:, dg : dg + 4, :], st_t[:, dg : dg + 4, :], mask_sb
                )
                # quad pre-reduction (each quad is 4 consecutive chunks),
                # then reduce quads to a single tile so Z is one matmul
                quads = []
                for qi in range(nkc // 4):
                    c0 = order[4 * qi]
                    p0 = ztp.tile([128, SB], BF, tag="zt")
                    nc.vector.tensor_add(p0, st_t[:, c0, :], st_t[:, c0 + 1, :])
                    p1 = ztp.tile([128, SB], BF, tag="zt")
                    nc.vector.tensor_add(
                        p1, st_t[:, c0 + 2, :], st_t[:, c0 + 3, :]
                    )
                    q0 = ztp.tile([128, SB], BF, tag="zt")
                    nc.vector.tensor_add(q0, p0, p1)
                    quads.append(q0)
                return st_t, quads

            def part2(b, qb, h, st_t, quads):
                """Z matmul, fast reciprocal, PV, at-scale for one group."""
                nkc = 4 * (qb + 1)
                qs0 = b * SEQ + qb * SB
                order = kc_order(qb, nkc)
                z_ps = psa.tile([128, SB], F32, tag="acc")
                o_ps = psa.tile([128, SB], F32, tag="acc")
                for i, q0 in enumerate(quads):
                    nc.tensor.matmul(
                        z_ps,
                        ones_sb,
                        q0,
                        start=(i == 0),
                        stop=(i == len(quads) - 1),
                    )
                zr_t = zrp.tile([128, SB], F32, tag="zr")
                nc.vector.reciprocal_approx_fast(zr_t, z_ps)
                for i, kc in enumerate(order):
                    nc.tensor.matmul(
                        o_ps,
                        vsb[:, b * (SEQ // 128) + kc, :],
                        st_t[:, kc, :],
                        start=(i == 0),
                        stop=(i == nkc - 1),
                    )
                nc.vector.tensor_mul(at[:, h, qs0 : qs0 + SB], o_ps, zr_t)

            def emit_wo(b, qb):
                for gcl in range(SB // 128):
                    gc = (b * SEQ + qb * SB) // 128 + gcl
                    for ob in range(DIM // SB):
                        op_ps = pjp.tile([128, SB], F32, tag="pj")
                        for jc in range(QH):
                            nc.tensor.matmul(
                                op_ps,
                                at[:, jc, gc * 128 : (gc + 1) * 128],
                                wo_sb[:, jc, ob * SB : (ob + 1) * SB],
                                start=(jc == 0),
                                stop=(jc == QH - 1),
                            )
                        ot = osp.tile([128, SB], BF, tag="os")
                        if ob == 3:
                            nc.scalar.copy(ot, op_ps)
                        else:
                            nc.vector.tensor_copy(ot, op_ps)
                        nc.sync.dma_start(
                            out[
                                gc * 128 : (gc + 1) * 128,
                                ob * SB : (ob + 1) * SB,
                            ],
                            ot,
                        )

            # ---- interleaved schedule ----
            P1 = "p1"
            seq_items = [
                (P1, 0),
                (P1, 1),
                (P1, 2),
                (P1, 3),
                (0, 0, 0),
                (0, 0, 1),
                (P1, 4),
                (0, 1, 0),
                (0, 1, 1),
                (P1, 5),
                (0, 2, 0),
                (0, 2, 1),
                (P1, 6),
                (0, 3, 0),
                (0, 3, 1),
                (P1, 7),
                (1, 0, 0),
                (1, 0, 1),
                (1, 3, 0),
                (1, 3, 1),
                (1, 2, 0),
                (1, 2, 1),
                (1, 1, 0),
                (1, 1, 1),
            ]

            prev = None
            for item in seq_items:
                if item[0] == P1:
                    sb = item[1]
                    phase1(sb)
                    if sb + 2 < NSB:
                        load_x(sb + 2)
                    continue
                b, qb, h = item
                st_t, quads = part1(b, qb, h)
                if prev is not None:
                    pb, pqb, ph, pst, pqs = prev
                    part2(pb, pqb, ph, pst, pqs)
                    if ph == QH - 1:
                        emit_wo(pb, pqb)
                prev = (b, qb, h, st_t, quads)
            pb, pqb, ph, pst, pqs = prev
            part2(pb, pqb, ph, pst, pqs)
            emit_wo(pb, pqb)

    nc.compile()
    return nc


def _prep_inputs(x, freqs_cos, freqs_sin, wq, wk, wv, wo):
    bf16 = ml_dtypes.bfloat16
    x2 = np.asarray(x, dtype=np.float32).reshape(S, DIM).T  # [DIM, S]
    # xR[p, sb, dc, s] = x2[dc*128+p, sb*SB+s]
    xR = np.ascontiguousarray(
        x2.reshape(NDC, 128, NSB, SB).transpose(1, 2, 0, 3)
    ).astype(bf16)
    cos = np.asarray(freqs_cos, np.float32).T  # [64, SEQ]
    sin = np.asarray(freqs_sin, np.float32).T
    cos2 = np.ascontiguousarray(np.concatenate([cos, cos], axis=0)).astype(bf16)
    sin2 = np.ascontiguousarray(np.concatenate([-sin, sin], axis=0)).astype(bf16)
    # maskw[p, r, q] = 1 iff q >= p + r*128
    pidx = np.arange(128)[:, None, None]
    ridx = np.arange(4)[None, :, None]
    qidx = np.arange(SB)[None, None, :]
    maskw = (qidx >= pidx + ridx * 128).astype(bf16)
    perm = np.concatenate([np.arange(0, HD, 2), np.arange(1, HD, 2)])
    scale = 1.0 / math.sqrt(HD)
    wq = np.asarray(wq, np.float32)
    wk = np.asarray(wk, np.float32)
    wv = np.asarray(wv, np.float32)
    wo = np.asarray(wo, np.float32)

    def part_major(wT, ncols):
        # wT: [DIM, ncols] -> [128, NDC, ncols]
        return np.ascontiguousarray(
            wT.reshape(NDC, 128, ncols).transpose(1, 0, 2)
        ).astype(bf16)

    in_maps = []
    for i in range(NCORES):
        wq_i = (wq[i * MQ : (i + 1) * MQ] * scale).reshape(QH, HD, DIM)[
            :, perm, :
        ].reshape(MQ, DIM)
        wk_i = wk[i * HD : (i + 1) * HD][perm]
        wv_i = wv[i * HD : (i + 1) * HD]
        wo_i = wo[:, i * MQ : (i + 1) * MQ]  # [DIM, MQ]
        woT = wo_i.T  # [MQ, DIM]
        woR = np.ascontiguousarray(
            woT.reshape(QH, 128, DIM).transpose(1, 0, 2)
        ).astype(bf16)
        in_maps.append(
            {
                "xR": xR,
                "wqR": part_major(np.ascontiguousarray(wq_i.T), MQ),
                "wkR": part_major(np.ascontiguousarray(wk_i.T), HD),
                "wvR": part_major(np.ascontiguousarray(wv_i.T), HD),
                "woR": woR,
                "cos2": cos2,
                "sin2": sin2,
                "maskw": maskw,
            }
        )
    return in_maps


def _run(inputs, trace=False):
    from concourse.bass_utils import run_bass_kernel_spmd

    if "nc" not in _CACHE:
        _CACHE["nc"] = _build()
    nc = _CACHE["nc"]
    in_maps = _prep_inputs(**inputs)
    res = run_bass_kernel_spmd(
        nc, in_maps, core_ids=list(range(NCORES)), trace=trace
    )
    partials = [np.asarray(r["out"], np.float32) for r in res.results]
    full = np.sum(partials, axis=0).reshape(BS, SEQ, DIM).astype(np.float32)
    return full, res


def kernel(**inputs):
    full, _ = _run(inputs, trace=False)
    return full


# revision 26
# speedup vs baseline: 1.2056x; 1.0236x over previous
"""Head-parallel GQA attention kernel for 8 TRN2 NeuronCores.

Sharding: core i owns KV head i and Q heads (2i, 2i+1), plus the matching
256-column slice of wo's input dim. Each core computes a partial output
(its heads' contribution through wo); the host sums the 8 partials.

All device compute is bf16 (PSUM accumulation in f32). The host pre-bakes
layouts so the device never rearranges inputs:
  - All DRAM inputs are partition-major [128, ...] with contiguous
    per-partition lines, so every load is ~128 fat descriptors.
  - wq/wk rows are permuted per-head (evens then odds) so interleaved-pair
    RoPE becomes rotate-half form: pairs live in partition halves [0:64]
    and [64:128] of the projected Q^T/K^T tiles. RoPE is then
    q' = R*C2 + rot(R)*S2 with C2=[cos;cos], S2=[-sin;sin] (3 full-width
    DVE ops + 2 ACT half-copies to build rot(R)).
  - 1/sqrt(head_dim) is folded into wq on the host.
  - Scores are computed transposed [ks, qs]; softmax needs no max
    subtraction (|S| <~ 12 for this data). Causal masking is one wide
    [128,4*512] multiply over the diagonal chunks. The denominator Z is a
    matmul with an all-ones stationary over DVE-pre-reduced quads, which
    also broadcasts Z across partitions for free; 1/Z uses the fast
    custom-DVE reciprocal. Normalization is folded into the PSUM->SBUF
    copy of the attention output.
  - Batch-0 attention groups interleave with batch-1 projection blocks so
    TensorE never drains; batch-1 groups run (0,3,2,1) so the tail is a
    small group.
"""

import math

import numpy as np
import ml_dtypes

BS, SEQ, DIM = 2, 2048, 2048
NH, NKV, HD = 16, 8, 128
S = BS * SEQ  # 4096
NCORES = 8
QH = NH // NCORES  # 2 q heads per core
MQ = QH * HD  # 256
SB = 512  # seq block
NSB = S // SB  # 8
NDC = DIM // 128  # 16 contraction chunks
QBLK = SEQ // SB  # 4 query blocks per batch
NKC_MAX = SEQ // 128  # 16

_CACHE = {}


def _build():
    import concourse.tile as tile
    from concourse import bacc, mybir

    BF = mybir.dt.bfloat16
    F32 = mybir.dt.float32
    Exp = mybir.ActivationFunctionType.Exp

    nc = bacc.Bacc(
        "TRN2", target_bir_lowering=False, debug=False, num_devices=NCORES
    )
    xR = nc.dram_tensor("xR", [128, NSB, NDC, SB], BF, kind="ExternalInput").ap()
    wqR = nc.dram_tensor("wqR", [128, NDC, MQ], BF, kind="ExternalInput").ap()
    wkR = nc.dram_tensor("wkR", [128, NDC, HD], BF, kind="ExternalInput").ap()
    wvR = nc.dram_tensor("wvR", [128, NDC, HD], BF, kind="ExternalInput").ap()
    woR = nc.dram_tensor("woR", [128, QH, DIM], BF, kind="ExternalInput").ap()
    cos2 = nc.dram_tensor("cos2", [128, SEQ], BF, kind="ExternalInput").ap()
    sin2 = nc.dram_tensor("sin2", [128, SEQ], BF, kind="ExternalInput").ap()
    maskw = nc.dram_tensor("maskw", [128, 128], BF, kind="ExternalInput").ap()
    out = nc.dram_tensor("out", [S, DIM], BF, kind="ExternalOutput").ap()

    with tile.TileContext(nc, pool_alloc_mode="queue") as tc:
        with tc.tile_pool(name="pers", bufs=1) as pers, tc.tile_pool(
            name="w1", bufs=1
        ) as w1p, tc.tile_pool(name="xt", bufs=2) as xtp, tc.tile_pool(
            name="rt", bufs=3
        ) as rtp, tc.tile_pool(name="vt", bufs=2) as vtp, tc.tile_pool(
            name="st", bufs=2
        ) as stp, tc.tile_pool(name="zt", bufs=12) as ztp, tc.tile_pool(name="zr", bufs=2) as zrp, tc.tile_pool(
            name="os", bufs=6
        ) as osp, tc.tile_pool(name="pj", bufs=3, space="PSUM") as pjp, tc.tile_pool(
            name="ps", bufs=3, space="PSUM"
        ) as psp, tc.tile_pool(name="acc", bufs=2, space="PSUM") as psa:
            qt = pers.tile([128, QH, S], BF, tag="qt")  # Q^T per head [hd, s]
            kt = pers.tile([128, S], BF, tag="kt")  # K^T [hd, s]
            vsb = pers.tile([128, S // 128, HD], BF, tag="v")  # V [s, vd]
            at = pers.tile([128, QH, S], BF, tag="at")  # attnout^T [vd, s]
            wo_sb = pers.tile([128, QH, DIM], BF, tag="wo")
            cos_sb = pers.tile([128, SEQ], BF, tag="cos")
            sin_sb = pers.tile([128, SEQ], BF, tag="sin")
            mask_sb = pers.tile([128, 128], BF, tag="mask")
            ones_sb = pers.tile([128, 128], BF, tag="ones")
            wq_sb = w1p.tile([128, NDC, MQ], BF, tag="wq")
            wk_sb = w1p.tile([128, NDC, HD], BF, tag="wk")
            wv_sb = w1p.tile([128, NDC, HD], BF, tag="wv")

            nc.vector.memset(ones_sb, 1.0)

            xt_tiles = {}

            def load_x(sb, split=False):
                t = xtp.tile([128, NDC, SB], BF, tag="xt")
                if split:
                    for c in range(4):
                        nc.sync.dma_start(
                            t[:, 4 * c : 4 * c + 4, :],
                            xR[:, sb, 4 * c : 4 * c + 4, :],
                        )
                else:
                    nc.sync.dma_start(t, xR[:, sb])
                xt_tiles[sb] = t

            # sync queue: wk then x stream; scalar queue: the rest.
            nc.sync.dma_start(wk_sb, wkR)
            load_x(0, split=True)
            nc.scalar.dma_start(wq_sb, wqR)
            nc.scalar.dma_start(wv_sb, wvR)
            load_x(1)
            nc.scalar.dma_start(cos_sb, cos2)
            nc.scalar.dma_start(sin_sb, sin2)
            nc.scalar.dma_start(mask_sb, maskw)
            nc.scalar.dma_start(wo_sb, woR)

            def phase1(sb):
                xt_t = xt_tiles.pop(sb)
                s0 = sb * SB
                seq0 = (sb % QBLK) * SB
                cs = cos_sb[:, seq0 : seq0 + SB]
                sn = sin_sb[:, seq0 : seq0 + SB]
                # K first (feeds scores soonest), then Q heads
                for which in (QH, 0, 1):
                    pst = pjp.tile([128, SB], F32, tag="pj")
                    for dc in range(NDC):
                        if which < QH:
                            lhs = wq_sb[:, dc, which * 128 : (which + 1) * 128]
                        else:
                            lhs = wk_sb[:, dc, :]
                        nc.tensor.matmul(
                            pst,
                            lhs,
                            xt_t[:, dc, :],
                            start=(dc == 0),
                            stop=(dc == NDC - 1),
                        )
                    if which < QH:
                        dest = qt[:, which, s0 : s0 + SB]
                    else:
                        dest = kt[:, s0 : s0 + SB]
                    rot = rtp.tile([128, SB], BF, tag="rot")
                    nc.scalar.copy(rot[64:128, :], pst[0:64, :])
                    nc.scalar.copy(rot[0:64, :], pst[64:128, :])
                    t1 = rtp.tile([128, SB], BF, tag="t1")
                    nc.vector.tensor_mul(t1, pst, cs)
                    t2 = rtp.tile([128, SB], BF, tag="t2")
                    nc.vector.tensor_mul(t2, rot, sn)
                    nc.vector.tensor_add(dest, t1, t2)
                # V natural [s, vd]
                for sc in range(SB // 128):
                    psv = pjp.tile([128, SB], F32, tag="pj")
                    pv0 = psv[:, 0:HD]
                    for dc in range(NDC):
                        nc.tensor.matmul(
                            pv0,
                            xt_t[:, dc, sc * 128 : (sc + 1) * 128],
                            wv_sb[:, dc, :],
                            start=(dc == 0),
                            stop=(dc == NDC - 1),
                        )
                    nc.vector.tensor_copy(vsb[:, sb * 4 + sc, :], pv0)

            def kc_order(qb, nkc):
                return list(range(4 * qb, nkc)) + list(range(4 * qb))

            def part1(b, qb, h):
                """Scores + exp + triangle mask + quad tree.

                Diagonal chunk r only touches its valid rectangle
                [r*128, SB); nothing downstream reads left of it, so SBUF
                slot reuse never exposes garbage."""
                nkc = 4 * (qb + 1)
                qs0 = b * SEQ + qb * SB
                order = kc_order(qb, nkc)
                st_t = stp.tile([128, NKC_MAX, SB], BF, tag="st")
                for kc in order:
                    r = kc - 4 * qb
                    off = r * 128 if r > 0 else 0
                    st_ps = psp.tile([128, SB], F32, tag="ps")
                    nc.tensor.matmul(
                        st_ps[:, off:SB],
                        kt[:, b * SEQ + kc * 128 : b * SEQ + (kc + 1) * 128],
                        qt[:, h, qs0 + off : qs0 + SB],
                        start=True,
                        stop=True,
                    )
                    nc.scalar.activation(
                        st_t[:, kc, off:SB], st_ps[:, off:SB], Exp
                    )
                # causal triangle lives only in the 128-col window at the
                # start of each diag chunk's rectangle (beyond it, q >= p+off
                # for all p); one shared [128,128] mask serves all four
                dg = 4 * qb
                for r in range(4):
                    off = r * 128
                    nc.vector.tensor_mul(
                        st_t[:, dg + r, off : off + 128],
                        st_t[:, dg + r, off : off + 128],
                        mask_sb,
                    )
                quads = []
                # diagonal quad: cascade of partial in-place adds so the
                # unwritten left regions are never read
                acc = ztp.tile([128, SB], BF, tag="zt")
                nc.vector.tensor_copy(acc, st_t[:, dg, :])
                for r in range(1, 4):
                    off = r * 128
                    nc.vector.tensor_add(
                        acc[:, off:SB], acc[:, off:SB], st_t[:, dg + r, off:SB]
                    )
                quads.append(acc)
                # rest quads (full chunks)
                for qi in range(1, nkc // 4):
                    c0 = order[4 * qi]
                    p0 = ztp.tile([128, SB], BF, tag="zt")
                    nc.vector.tensor_add(p0, st_t[:, c0, :], st_t[:, c0 + 1, :])
                    p1 = ztp.tile([128, SB], BF, tag="zt")
                    nc.vector.tensor_add(
                        p1, st_t[:, c0 + 2, :], st_t[:, c0 + 3, :]
                    )
                    q0 = ztp.tile([128, SB], BF, tag="zt")
                    nc.vector.tensor_add(q0, p0, p1)
                    quads.append(q0)
                return st_t, quads

            def part2(b, qb, h, st_t, quads):
                """Z matmul, fast reciprocal, PV, at-scale for one group."""
                nkc = 4 * (qb + 1)
                qs0 = b * SEQ + qb * SB
                order = kc_order(qb, nkc)
                z_ps = psa.tile([128, SB], F32, tag="acc")
                o_ps = psa.tile([128, SB], F32, tag="acc")
                for i, q0 in enumerate(quads):
                    nc.tensor.matmul(
                        z_ps,
                        ones_sb,
                        q0,
                        start=(i == 0),
                        stop=(i == len(quads) - 1),
                    )
                zr_t = zrp.tile([128, SB], F32, tag="zr")
                nc.vector.reciprocal_approx_fast(zr_t, z_ps)
                for i, kc in enumerate(order):
                    # diagonal chunks contribute zero left of their valid
                    # rectangle (masked), so accumulate only [off:SB); the
                    # first matmul (r==0) covers the full bank
                    r = kc - 4 * qb
                    off = r * 128 if r > 0 else 0
                    nc.tensor.matmul(
                        o_ps[:, off:SB],
                        vsb[:, b * (SEQ // 128) + kc, :],
                        st_t[:, kc, off:SB],
                        start=(i == 0),
                        stop=(i == nkc - 1),
                    )
                nc.vector.tensor_mul(at[:, h, qs0 : qs0 + SB], o_ps, zr_t)

            def emit_wo(b, qb):
                for gcl in range(SB // 128):
                    gc = (b * SEQ + qb * SB) // 128 + gcl
                    for ob in range(DIM // SB):
                        op_ps = pjp.tile([128, SB], F32, tag="pj")
                        for jc in range(QH):
                            nc.tensor.matmul(
                                op_ps,
                                at[:, jc, gc * 128 : (gc + 1) * 128],
                                wo_sb[:, jc, ob * SB : (ob + 1) * SB],
                                start=(jc == 0),
                                stop=(jc == QH - 1),
                            )
                        ot = osp.tile([128, SB], BF, tag="os")
                        if ob % 2:
                            nc.scalar.copy(ot, op_ps)
                        else:
                            nc.vector.tensor_copy(ot, op_ps)
                        nc.sync.dma_start(
                            out[
                                gc * 128 : (gc + 1) * 128,
                                ob * SB : (ob + 1) * SB,
                            ],
                            ot,
                        )

            # ---- interleaved schedule ----
            P1 = "p1"
            seq_items = [
                (P1, 0),
                (P1, 1),
                (P1, 2),
                (P1, 3),
                (0, 0, 0),
                (0, 0, 1),
                (P1, 4),
                (0, 1, 0),
                (0, 1, 1),
                (P1, 5),
                (0, 2, 0),
                (0, 2, 1),
                (P1, 6),
                (0, 3, 0),
                (0, 3, 1),
                (P1, 7),
                (1, 0, 0),
                (1, 0, 1),
                (1, 3, 0),
                (1, 3, 1),
                (1, 2, 0),
                (1, 2, 1),
                (1, 1, 0),
                (1, 1, 1),
            ]

            prev = None
            for item in seq_items:
                if item[0] == P1:
                    sb = item[1]
                    phase1(sb)
                    if sb + 2 < NSB:
                        load_x(sb + 2)
                    continue
                b, qb, h = item
                st_t, quads = part1(b, qb, h)
                if prev is not None:
                    pb, pqb, ph, pst, pqs = prev
                    part2(pb, pqb, ph, pst, pqs)
                    if ph == QH - 1:
                        emit_wo(pb, pqb)
                prev = (b, qb, h, st_t, quads)
            pb, pqb, ph, pst, pqs = prev
            part2(pb, pqb, ph, pst, pqs)
            emit_wo(pb, pqb)

    nc.compile()
    return nc


def _prep_inputs(x, freqs_cos, freqs_sin, wq, wk, wv, wo):
    bf16 = ml_dtypes.bfloat16
    x2 = np.asarray(x, dtype=np.float32).reshape(S, DIM).T  # [DIM, S]
    # xR[p, sb, dc, s] = x2[dc*128+p, sb*SB+s]
    xR = np.ascontiguousarray(
        x2.reshape(NDC, 128, NSB, SB).transpose(1, 2, 0, 3)
    ).astype(bf16)
    cos = np.asarray(freqs_cos, np.float32).T  # [64, SEQ]
    sin = np.asarray(freqs_sin, np.float32).T
    cos2 = np.ascontiguousarray(np.concatenate([cos, cos], axis=0)).astype(bf16)
    sin2 = np.ascontiguousarray(np.concatenate([-sin, sin], axis=0)).astype(bf16)
    # shared causal triangle: maskw[p, j] = 1 iff j >= p
    maskw = (np.arange(128)[None, :] >= np.arange(128)[:, None]).astype(bf16)
    perm = np.concatenate([np.arange(0, HD, 2), np.arange(1, HD, 2)])
    scale = 1.0 / math.sqrt(HD)
    wq = np.asarray(wq, np.float32)
    wk = np.asarray(wk, np.float32)
    wv = np.asarray(wv, np.float32)
    wo = np.asarray(wo, np.float32)

    def part_major(wT, ncols):
        # wT: [DIM, ncols] -> [128, NDC, ncols]
        return np.ascontiguousarray(
            wT.reshape(NDC, 128, ncols).transpose(1, 0, 2)
        ).astype(bf16)

    in_maps = []
    for i in range(NCORES):
        wq_i = (wq[i * MQ : (i + 1) * MQ] * scale).reshape(QH, HD, DIM)[
            :, perm, :
        ].reshape(MQ, DIM)
        wk_i = wk[i * HD : (i + 1) * HD][perm]
        wv_i = wv[i * HD : (i + 1) * HD]
        wo_i = wo[:, i * MQ : (i + 1) * MQ]  # [DIM, MQ]
        woT = wo_i.T  # [MQ, DIM]
        woR = np.ascontiguousarray(
            woT.reshape(QH, 128, DIM).transpose(1, 0, 2)
        ).astype(bf16)
        in_maps.append(
            {
                "xR": xR,
                "wqR": part_major(np.ascontiguousarray(wq_i.T), MQ),
                "wkR": part_major(np.ascontiguousarray(wk_i.T), HD),
                "wvR": part_major(np.ascontiguousarray(wv_i.T), HD),
                "woR": woR,
                "cos2": cos2,
                "sin2": sin2,
                "maskw": maskw,
            }
        )
    return in_maps


def _run(inputs, trace=False):
    from concourse.bass_utils import run_bass_kernel_spmd

    if "nc" not in _CACHE:
        _CACHE["nc"] = _build()
    nc = _CACHE["nc"]
    in_maps = _prep_inputs(**inputs)
    res = run_bass_kernel_spmd(
        nc, in_maps, core_ids=list(range(NCORES)), trace=trace
    )
    partials = [np.asarray(r["out"], np.float32) for r in res.results]
    full = np.sum(partials, axis=0).reshape(BS, SEQ, DIM).astype(np.float32)
    return full, res


def kernel(**inputs):
    full, _ = _run(inputs, trace=False)
    return full
